# revision 140
# baseline (speedup 1.0000x reference)
"""Trainium2 Bass kernel for the RBF-mixture value network (retrieval_knn).

Math (per batch row b):
    values  = MLP_relu3(s) @ Wv4 + bv4                      [N]
    h       = relu(s @ Wl1)                                 [H]
    cent    = tanh(h @ Wg + bexp)      (Wg = Wexp^T flat)   [N*A]
    dist[n] = sqrt(sum_a (cent[n,a] - a_vec[a])^2 + 0.01)
    out     = sum_n softmax(-dist)[n] * values[n]

Sharding: pure data-parallel over B across 8 cores (512 rows each), all
parameters replicated; no collectives.

v2 layout (vs the v1 [b-part, na-free] kernel):
  * the expert einsum runs in fp8(e4m3) with DoubleRow perf mode: lhsT is a
    [K=128, 2, 128] slice of the pre-scaled fp8 weights, rhs is the fp8
    hidden state [K=128, 2, 512]; each matmul covers TWO K-planes.
  * einsum output is [na-part, b-free]: partition p of na-tile t is expert
    n=4t+p//32, coord a=p%32.  In this layout bexp is a per-partition scalar,
    so it rides the tanh's bias operand for free (scale 2^-17 de-scales fp8).
  * the sum over a (32 partitions) is a PE mask-matmul: 32 accumulating
    matmuls per 128-expert bank land dist^2 directly in PSUM as [n, b].
  * softmax reductions over n are PE ones-matmuls; MLP relu+bias run on DVE
    tensor_scalar ops to keep ACT free for the 64-tile tanh stream.
  * host-side: bexp is pre-corrected by the mean fp8 quantization error
    (mu_h @ dW), recovering ~7% of the fp8 error.
"""

from contextlib import ExitStack

import numpy as np
import ml_dtypes

import concourse.bacc as bacc
import concourse.bass as bass
import concourse.mybir as mybir
import concourse.tile as tile
from concourse.bass import ts
from concourse.bass_utils import run_bass_kernel_spmd

BF16 = mybir.dt.bfloat16
FP8 = mybir.dt.float8e4
F32 = mybir.dt.float32
AF = mybir.ActivationFunctionType
ALU = mybir.AluOpType
PM = mybir.MatmulPerfMode

B, S, A, H, N = 4096, 128, 32, 1024, 256
NCORES = 8
BL = B // NCORES          # 512 rows per core
KT = H // 128             # 8 contraction tiles
NA = N * A                # 8192
NAT = NA // 128           # 64 einsum partition-tiles
TM = 32                   # masks (one per na-tile within a 128-expert bank)
NB = N // 128             # 2 expert banks
NORM_SMOOTHING = 0.01
SH = 64.0                 # fp8 scale for hidden state
SW = 2048.0               # fp8 scale for einsum weights
SINV = 1.0 / (SH * SW)    # de-scale folded into the tanh
SA = 16.0                 # fp8 activation scale for the value MLP
SCW = 4096.0              # fp8 weight scale for Wv2/Wv3

_prog_cache = {}


def _tct(tc, stk, shape, dtype, name, space="SBUF"):
    t, free = tc.tile(shape, dtype, name=name, space=space)
    stk.callback(free)
    return t


def _build_program():
    nc = bacc.Bacc(None, target_bir_lowering=False)

    # ---- DRAM I/O (per-core shapes) ----
    d_swl = nc.dram_tensor("swl", [128, BL + H], BF16, kind="ExternalInput")
    d_wv1 = nc.dram_tensor("wv1", [128, H], BF16, kind="ExternalInput")
    d_wv2hi = nc.dram_tensor("wv2hi", [128, KT, H], FP8, kind="ExternalInput")
    d_wv2lo = nc.dram_tensor("wv2lo", [128, KT, H], FP8, kind="ExternalInput")
    d_wv3hi = nc.dram_tensor("wv3hi", [128, KT, H], FP8, kind="ExternalInput")
    d_wv3lo = nc.dram_tensor("wv3lo", [128, KT, H], FP8, kind="ExternalInput")
    d_wv4T = nc.dram_tensor("wv4T", [128, KT, NB, 128], BF16, kind="ExternalInput")
    d_wg8 = nc.dram_tensor("wg8", [128, NAT, KT, 128], FP8, kind="ExternalInput")
    d_aT4 = nc.dram_tensor("aT4", [128, BL], BF16, kind="ExternalInput")
    d_masks = nc.dram_tensor("masks", [128, TM, 128], FP8, kind="ExternalInput")
    d_bexpT = nc.dram_tensor("bexpT", [128, NAT], F32, kind="ExternalInput")
    d_biasT = nc.dram_tensor("biasT", [128, 3, KT], F32, kind="ExternalInput")
    d_bv4T = nc.dram_tensor("bv4T", [128, NB], F32, kind="ExternalInput")
    d_out = nc.dram_tensor("out", [33, BL], F32, kind="ExternalOutput")

    with tile.TileContext(nc) as tc, ExitStack() as stk:
        # ---- persistent SBUF ----
        swl = _tct(tc, stk, [128, BL + H], BF16, "swl_sb")
        sT = swl[:, :BL]
        wl1 = swl[:, BL:].rearrange("p (k m) -> p k m", m=128)
        aT4 = _tct(tc, stk, [128, BL], BF16, "aT4_sb")
        wv1 = _tct(tc, stk, [128, H], BF16, "wv1_sb")
        wv2hi = _tct(tc, stk, [128, KT, H], FP8, "wv2hi_sb")
        wv2lo = _tct(tc, stk, [128, KT, H], FP8, "wv2lo_sb")
        wv3hi = _tct(tc, stk, [128, KT, H], FP8, "wv3hi_sb")
        wv3lo = _tct(tc, stk, [128, KT, H], FP8, "wv3lo_sb")
        wv4T = _tct(tc, stk, [128, KT, NB, 128], BF16, "wv4T_sb")
        masks = _tct(tc, stk, [128, TM, 128], FP8, "masks_sb")
        bexpT = _tct(tc, stk, [128, NAT], F32, "bexpT_sb")
        biasT = _tct(tc, stk, [128, 3, KT], F32, "biasT_sb")
        bv4T = _tct(tc, stk, [128, NB], F32, "bv4T_sb")

        HT8 = [_tct(tc, stk, [128, 2, BL], FP8, f"HT8_{i}") for i in range(4)]
        T1hi = _tct(tc, stk, [128, KT, BL], FP8, "T1hi_sb")
        T1lo = _tct(tc, stk, [128, KT, BL], FP8, "T1lo_sb")
        T2hi = _tct(tc, stk, [128, KT, BL], FP8, "T2hi_sb")
        T2lo = _tct(tc, stk, [128, KT, BL], FP8, "T2lo_sb")
        T3 = _tct(tc, stk, [128, KT, BL], BF16, "T3_sb")
        dist = _tct(tc, stk, [128, NB, BL], F32, "dist_sb")
        E = _tct(tc, stk, [128, NB, BL], BF16, "E_sb")
        V = _tct(tc, stk, [128, NB, BL], BF16, "V_sb")
        EV = _tct(tc, stk, [128, NB, BL], BF16, "EV_sb")
        ones1 = _tct(tc, stk, [128, 1], BF16, "ones1_sb")
        smooth = _tct(tc, stk, [128, 1], F32, "smooth_sb")
        osb = _tct(tc, stk, [33, BL], F32, "osb_sb")
        nc.vector.memset(ones1[:], 1.0)
        nc.vector.memset(smooth[:], NORM_SMOOTHING)

        # persistent PSUM: dist^2 accumulator banks (also reused for den/num)
        dps = [
            _tct(tc, stk, [128, BL], F32, f"dist2_ps{b}", space="PSUM")
            for b in range(NB)
        ]

        # ---- prologue DMAs (order = DMA-engine service order; wl1 halves
        # first so the L1 pipeline starts ~700ns earlier; einsum-phase
        # operands are interleaved with the first wg chunks further below) ----
        nc.sync.dma_start(out=swl[:, : BL + H // 2], in_=d_swl[:][:, : BL + H // 2])
        nc.sync.dma_start(out=swl[:, BL + H // 2 :], in_=d_swl[:][:, BL + H // 2 :])
        # keep PE continuously busy until the first real matmul so the
        # p-state ramp is warm when wl1/sT land (dummy reads of zeroed SBUF)
        for _ in range(3):
            nc.tensor.matmul(dps[1][0:1, :], ones1[:], aT4[:, :], start=True, stop=True)

        wg2_pool = stk.enter_context(tc.tile_pool(name="wg2_pool", bufs=2))
        wg4_pool = stk.enter_context(tc.tile_pool(name="wg4_pool", bufs=1))
        wg8_pool = stk.enter_context(tc.tile_pool(name="wg8_pool", bufs=4))
        c_pool = stk.enter_context(tc.tile_pool(name="c_pool", bufs=8))
        d_pool = stk.enter_context(tc.tile_pool(name="d_pool", bufs=8))
        d2_pool = stk.enter_context(tc.tile_pool(name="d2_pool", bufs=8))
        ts_pool = stk.enter_context(tc.tile_pool(name="ts_pool", bufs=4))
        ps_ein = stk.enter_context(tc.tile_pool(name="ps_ein", bufs=4, space="PSUM"))
        ps_mlp = stk.enter_context(tc.tile_pool(name="ps_mlp", bufs=2, space="PSUM"))

        # ---- L1 location: HT8 = fp8(SH * relu(s @ Wl1)), bl1 == 0 ----
        # alternate epilogues between ACT and DVE: the 2-bank psum pipeline
        # paces on the epilogue engine while PE waits for HT8
        # L1 psums come from the (otherwise idle) 4-deep ps_ein pool and the
        # relu drain spreads over ACT/DVE/Pool: HT8 gates the whole einsum
        for j in range(KT):
            if j >= 6:
                ps = dps[j - 6]
            else:
                ps = ps_ein.tile([128, BL], F32, tag="ps_ein", name="ps_l1l")
            nc.tensor.matmul(ps[:], wl1[:, j, :], sT[:], start=True, stop=True)
            # NOTE: GPSIMD cannot read PSUM, so only ACT/DVE drain these
            out = HT8[j // 2][:, j % 2, :]
            if j % 2 == 0:
                nc.scalar.activation(out, ps[:], AF.Relu, scale=SH)
            else:
                nc.vector.tensor_scalar(out, ps[:], 0.0, SH, ALU.max, ALU.mult)

        def split_hilo(ts_f32, Thi, Tlo, j):
            # hi cast + residual both on the otherwise-idle Pool engine
            nc.gpsimd.tensor_copy(Thi[:, j, :], ts_f32[:])
            nc.gpsimd.tensor_sub(Tlo[:, j, :], ts_f32[:], Thi[:, j, :])

        def l1v_step(j):
            # L1 value (T1s = SA*relu(s @ Wv1 + bv1); Wv1/bv1 pre-scaled by
            # SA on host) interleaved into the first einsum steps
            ps = ps_mlp.tile([128, BL], F32, tag="ps_mlp", name="ps_l1v")
            nc.tensor.matmul(ps[:], wv1[:, ts(j, 128)], sT[:], start=True, stop=True)
            t1s = ts_pool.tile([128, BL], F32, tag="ts", name="t1s")
            nc.vector.tensor_scalar(
                t1s[:], ps[:], biasT[:, 0, j : j + 1], 0.0, ALU.add, ALU.max
            )
            split_hilo(t1s, T1hi, T1lo, j)

        # scheduled extra work per einsum step ------------------------------
        mlp_ps = {}  # open accumulation tiles for interleaved layers

        def hh_partial(layer_i, Whi, Wlo, Tinhi, Tinlo, epilogue):
            # one j-slice of an H->H layer as 3 fp8 DoubleRow groups
            # (hi@hi + hi-W@lo-act + lo-W@hi-act), 3 matmuls per step over 4
            # steps; flat order delays the first Wlo/Tinlo use by 1-2 steps so
            # their DMAs/producers get extra slack
            flat = (
                [(Whi, Tinhi, kp) for kp in range(4)]
                + [(Whi, Tinlo, kp) for kp in range(4)]
                + [(Wlo, Tinhi, kp) for kp in range(4)]
            )

            def emit(dk, j):
                if dk == 0:
                    mlp_ps[layer_i] = ps_mlp.tile(
                        [128, BL], F32, tag="ps_mlp", name="ps_hh"
                    )
                ps = mlp_ps[layer_i]
                for i in range(3 * dk, 3 * dk + 3):
                    W, Tin, kp = flat[i]
                    sl = slice(2 * kp, 2 * kp + 2)
                    nc.tensor.matmul(
                        ps[:], W[:, sl, ts(j, 128)], Tin[:, sl, :],
                        start=(i == 0),
                        stop=(i == 11),
                        perf_mode=PM.DoubleRow,
                    )
                if dk == 3:
                    epilogue(ps, j)
            return emit

        def l2_epilogue(ps, j):
            # T2s = SCW-descale relu (stays SA-scaled); hi cast on ACT (slack)
            # so Pool's queue only carries the residual subs
            t2s = ts_pool.tile([128, BL], F32, tag="ts", name="t2s")
            nc.scalar.activation(
                t2s[:], ps[:], AF.Relu, bias=biasT[:, 1, j : j + 1], scale=1.0 / SCW
            )
            nc.scalar.copy(T2hi[:, j, :], t2s[:])
            nc.gpsimd.tensor_sub(T2lo[:, j, :], t2s[:], T2hi[:, j, :])

        def l3_epilogue(ps, j):
            nc.scalar.activation(
                T3[:, j, :], ps[:], AF.Relu, bias=biasT[:, 2, j : j + 1],
                scale=1.0 / (SA * SCW),
            )

        sched = {}

        def at(t, fn):
            sched.setdefault(t, []).append(fn)

        # L1-value interleaves into steps 0..7; late weight DMAs ride the wg
        # chunk stream (emission order = DMA service order, tuned by need)
        for j in range(KT):
            at(j, lambda j=j: l1v_step(j))
        def half_dma(dst, src, h):
            nc.sync.dma_start(
                out=dst[:, 4 * h : 4 * h + 4], in_=src[:][:, 4 * h : 4 * h + 4]
            )

        at(6, lambda: half_dma(wv2hi, d_wv2hi, 0))
        at(7, lambda: half_dma(wv2hi, d_wv2hi, 1))
        at(10, lambda: half_dma(wv2lo, d_wv2lo, 0))
        at(11, lambda: half_dma(wv2lo, d_wv2lo, 1))
        at(18, lambda: half_dma(wv3hi, d_wv3hi, 0))
        at(19, lambda: half_dma(wv3hi, d_wv3hi, 1))
        at(22, lambda: half_dma(wv3lo, d_wv3lo, 0))
        at(23, lambda: half_dma(wv3lo, d_wv3lo, 1))
        at(42, lambda: nc.sync.dma_start(out=wv4T[:], in_=d_wv4T[:]))

        # L2 over steps 16..47, L3 over 48..63
        for j in range(KT):
            emit = hh_partial(1, wv2hi, wv2lo, T1hi, T1lo, l2_epilogue)
            for dk in range(4):
                at(18 + 4 * j + dk, lambda e=emit, dk=dk, j=j: e(dk, j))
        for j in range(KT):
            emit = hh_partial(2, wv3hi, wv3lo, T2hi, T2lo, l3_epilogue)
            for dk in range(4):
                at(54 + 2 * j + dk // 2, lambda e=emit, dk=dk, j=j: e(dk, j))

        def v_full(nb):
            ps = ps_mlp.tile([128, BL], F32, tag="ps_mlp", name="ps_v")
            for k in range(KT):
                nc.tensor.matmul(
                    ps[:], wv4T[:, k, nb, :], T3[:, k, :],
                    start=(k == 0), stop=(k == KT - 1),
                )
            nc.vector.tensor_scalar(
                V[:, nb, :], ps[:], bv4T[:, nb : nb + 1], None, ALU.add
            )



        def finish_bank(b):
            # dist -> E; sqrt forces an ACT table switch and exp switches back
            # to the tanh table (pow/divide have no DVE/GPSIMD lowering)
            nc.scalar.activation(dist[:, b, :], dps[b][:], AF.Sqrt, bias=smooth[:, 0:1])
            nc.scalar.activation(E[:, b, :], dist[:, b, :], AF.Exp, scale=-1.0)

        # both banks finish in the tail: sqrt mid-stream would cost two extra
        # table switches (4 total); tail-only needs exactly 2

        # ---- einsum + distance pipeline ----
        # wg streams in chunks: small chunks prime the pipe fast, then 8-tile
        # chunks (8 KB/partition per DMA); the remaining prologue operands
        # ride between the first chunks in need order
        bounds = [0, 2, 4, 8, 16, 24, 32, 40, 48, 56, 64]
        chunk_of = {}
        for c in range(len(bounds) - 1):
            for t in range(bounds[c], bounds[c + 1]):
                chunk_of[t] = c
        chunks = {}

        def fetch(c):
            if c < len(bounds) - 1:
                n = bounds[c + 1] - bounds[c]
                pool = {2: wg2_pool, 4: wg4_pool, 8: wg8_pool}[n]
                wgc = pool.tile([128, n, KT, 128], FP8, tag=f"wgc{n}", name="wgc")
                nc.sync.dma_start(out=wgc[:], in_=d_wg8[:][:, bounds[c] : bounds[c + 1]])
                chunks[c] = wgc

        fetch(0)
        nc.sync.dma_start(out=wv1[:], in_=d_wv1[:])
        fetch(1)
        nc.sync.dma_start(out=biasT[:], in_=d_biasT[:])
        nc.sync.dma_start(out=bv4T[:], in_=d_bv4T[:])
        nc.sync.dma_start(out=bexpT[:], in_=d_bexpT[:])
        nc.sync.dma_start(out=aT4[:], in_=d_aT4[:])
        nc.sync.dma_start(out=masks[:, : TM // 2], in_=d_masks[:][:, : TM // 2])
        fetch(2)
        nc.sync.dma_start(out=masks[:, TM // 2 :], in_=d_masks[:][:, TM // 2 :])
        fetch(3)

        for t in range(NAT):
            c = chunk_of[t]
            if c + 1 <= 9 and c >= 0 and t == bounds[c + 1] - 1:
                chunks.pop(c - 1, None)
                fetch(c + 4)
            wgc = chunks[c]
            ps = ps_ein.tile([128, BL], F32, tag="ps_ein")
            for kp in range(4):
                nc.tensor.matmul(
                    ps[:],
                    wgc[:, t - bounds[c], 2 * kp : 2 * kp + 2, :],
                    HT8[kp][:],
                    start=(kp == 0),
                    stop=(kp == 3),
                    perf_mode=PM.DoubleRow,
                )
            C = c_pool.tile([128, BL], BF16, tag="C")
            nc.scalar.activation(C[:], ps[:], AF.Tanh, bias=bexpT[:, t : t + 1], scale=SINV)
            D = d_pool.tile([128, BL], BF16, tag="D")
            nc.vector.tensor_sub(D[:], C[:], aT4[:])
            D2 = d2_pool.tile([128, BL], BF16, tag="D2")
            nc.vector.tensor_mul(D2[:], D[:], D[:])
            nc.tensor.matmul(
                dps[t // TM][:], masks[:, t % TM, :], D2[:],
                start=(t % TM == 0), stop=(t % TM == TM - 1),
            )
            for fn in sched.get(t, ()):
                fn()

        for t2 in range(NAT, max(sched) + 1):
            for fn in sched.get(t2, ()):
                fn()

        # ---- tail: V^T while ACT switches tables; wide sqrt/exp over both
        # banks at once, then the weighted sum ----
        for nb in range(NB):
            v_full(nb)
        for b in range(NB):
            nc.scalar.activation(dist[:, b, :], dps[b][:], AF.Sqrt, bias=smooth[:, 0:1])
        nc.scalar.activation(E[:, 0, :], dist[:, 0, :], AF.Exp, scale=-1.0)
        nc.tensor.matmul(dps[0][0:1, :], ones1[:], E[:, 0, :], start=True, stop=False)
        nc.vector.tensor_mul(EV[:, 0, :], E[:, 0, :], V[:, 0, :])
        nc.tensor.matmul(dps[0][32:33, :], ones1[:], EV[:, 0, :], start=True, stop=False)
        nc.scalar.activation(E[:, 1, :], dist[:, 1, :], AF.Exp, scale=-1.0)
        nc.tensor.matmul(dps[0][0:1, :], ones1[:], E[:, 1, :], start=False, stop=True)
        nc.vector.tensor_mul(EV[:, 1, :], E[:, 1, :], V[:, 1, :])
        nc.tensor.matmul(dps[0][32:33, :], ones1[:], EV[:, 1, :], start=False, stop=True)
        nc.vector.tensor_copy(osb[:], dps[0][0:33, :])
        nc.sync.dma_start(out=d_out[:], in_=osb[:])

    nc.finalize()
    return nc


def _bf16(x):
    return np.ascontiguousarray(x.astype(ml_dtypes.bfloat16))


def _prepare_in_maps(s, a, Wv1, bv1, Wv2, bv2, Wv3, bv3, Wv4, bv4, Wl1, bl1, Wexp, bexp):
    s = np.asarray(s, np.float32)
    a = np.asarray(a, np.float32)
    f32 = np.float32

    wl1_f = np.asarray(Wl1, f32)
    wv1 = _bf16(np.asarray(Wv1, f32) * SA)
    wl1 = _bf16(wl1_f)

    def _hilo8(W):
        Ws = np.asarray(W, f32) * SCW
        hi = Ws.astype(ml_dtypes.float8_e4m3)
        lo = (Ws - hi.astype(f32)).astype(ml_dtypes.float8_e4m3)
        perm = lambda x: np.ascontiguousarray(
            x.reshape(KT, 128, H).transpose(1, 0, 2)
        )
        return perm(hi), perm(lo)

    wv2hi, wv2lo = _hilo8(Wv2)
    wv3hi, wv3lo = _hilo8(Wv3)
    wv4T = _bf16(np.asarray(Wv4, f32).reshape(KT, 128, NB, 128).transpose(1, 0, 2, 3))

    wg_full = np.asarray(Wexp, f32).transpose(1, 0, 2).reshape(H, NA)
    w8 = (wg_full * SW).astype(ml_dtypes.float8_e4m3)
    wg8 = np.ascontiguousarray(
        w8.reshape(KT, 128, NAT, 128).transpose(1, 2, 0, 3)
    )

    # fold the mean fp8 weight-quantization error into bexp:
    # corr[na] = mean_b(h8/SH)[k] @ (w8/SW - Wg)[k, na]
    h = np.maximum(_bf16(s).astype(f32) @ _bf16(wl1_f).astype(f32), 0.0)
    h8 = (h * SH).astype(ml_dtypes.float8_e4m3).astype(f32)
    mu_h = (h8 / SH).mean(axis=0)
    dW = w8.astype(f32) / SW - wg_full
    corr = mu_h @ dW
    bexp_adj = np.asarray(bexp, f32).reshape(NA) - corr
    bexpT = np.ascontiguousarray(bexp_adj.reshape(NAT, 128).T.astype(f32))

    p = np.arange(128)
    tm = np.arange(TM)
    m = np.arange(128)
    masks = np.ascontiguousarray(
        (m[None, None, :] == (4 * tm[None, :, None] + p[:, None, None] // 32)).astype(
            ml_dtypes.float8_e4m3
        )
    )

    biasT = np.ascontiguousarray(
        np.stack(
            [SA * np.asarray(bv1, f32), SA * np.asarray(bv2, f32), np.asarray(bv3, f32)]
        )
        .reshape(3, KT, 128)
        .transpose(2, 0, 1)
        .astype(f32)
    )
    bv4T = np.ascontiguousarray(np.asarray(bv4, f32).reshape(NB, 128).T.astype(f32))

    in_maps = []
    for c in range(NCORES):
        rows = slice(c * BL, (c + 1) * BL)
        swl = _bf16(np.concatenate([s[rows].T.astype(f32), wl1_f], axis=1))
        aT4 = _bf16(np.tile(a[rows].T, (4, 1)))
        in_maps.append(
            dict(
                swl=swl, aT4=aT4, wv1=wv1, wv2hi=wv2hi, wv2lo=wv2lo,
                wv3hi=wv3hi, wv3lo=wv3lo, wv4T=wv4T,
                wg8=wg8, masks=masks, bexpT=bexpT, biasT=biasT, bv4T=bv4T,
            )
        )
    return in_maps


def _run(inputs, trace=False, **trace_kwargs):
    if "nc" not in _prog_cache:
        _prog_cache["nc"] = _build_program()
    nc = _prog_cache["nc"]
    in_maps = _prepare_in_maps(**inputs)
    res = run_bass_kernel_spmd(
        nc, in_maps, core_ids=list(range(NCORES)), trace=trace, **trace_kwargs
    )
    out = np.concatenate(
        [(r["out"][32] / r["out"][0]).reshape(BL) for r in res.results]
    ).reshape(B, 1).astype(np.float32)
    return out, res


def kernel(**inputs) -> np.ndarray:
    out, _ = _run(inputs)
    return out
